# revision 62
# baseline (speedup 1.0000x reference)
"""Trainium2 Bass kernel for DemandAwareCrossAttention.

Reference computation (per pixel, fully pointwise in (H, W)):
    enc  = w_d2 @ relu(w_d1 @ demand + b_d1) + b_d2
    qs   = ego + enc + pos
    q    = (wq @ qs + bq)   reshaped [8 heads, 32]
    k_n  = wk @ collab_n + bk ; v_n = wv @ collab_n + bv     (n = 0..3)
    s_ng = q_g . k_ng / sqrt(32)
    a    = softmax_n(s)
    u    = sum_n a_ng * v_n            -> [256]
    out  = wo @ u + bo

Sharding: split H across the 8 cores (16 rows = 4096 pixels each); weights
replicated.  pos_emb is folded into ego on the host; bk cancels in the
softmax and is dropped; wq@b_d2+bq rides the q copy; wo@bv+bo rides the
output copy; q is pre-scaled by 1/sqrt(32) on host.

Device layout: channels on SBUF partitions, pixels on the free dim, channel
chunks c in {0,1} of 128.  One 256-pixel tile per H row.  Per tile:
  - ego + 4 collabs arrive as ONE contiguous DMA ([128, 5, 2, 256] bf16)
  - all 1x1 convs are PE matmuls (bf16, fp32 PSUM accumulate); the k/v
    projections are emitted in lhsT-grouped runs so the PE streams
    back-to-back (keeps the HAM clock gate at 2.4 GHz)
  - scores: DVE q*k product, then masked matmuls pack score(n, head g) at
    PSUM row 32n+8+g of a [128, 256] tile (both chunks' heads in one
    256-wide tile - halves the free dim every softmax op pays for)
  - divide-free softmax: e = exp(s) (ScalarE), denom via masked matmul
    into rows 0..7, L = ln(denom) into the copy's rows 0..7, then masked
    matmuls form z = s - L broadcast over head_dim, a = exp(z)
  - combine u = sum_n a_n * v_n on GpSimd (muls + adds); q/out PSUM->SBUF
    copies (+ bias) on DVE; exp/ln stay on ScalarE
  - output stored as bf16 (halves the writeback)

Engine budget per tile (target):  PE ~59 matmuls of 256 cols; ScalarE
exp/copy/ln + 4 big exps; DVE q-copy + 4 muls + o-copy; GpSimd 4 muls +
3 adds; one DMA in, one out.
"""

import math
import numpy as np
import ml_dtypes
from contextlib import ExitStack

import concourse.bass as bass
import concourse.tile as tile
from concourse import bacc, mybir
from concourse.bass import ts
from concourse.bass_utils import run_bass_kernel_spmd

BF = mybir.dt.bfloat16
F32 = mybir.dt.float32
AF = mybir.ActivationFunctionType

# All ScalarE functions used here (Exp/Ln/Relu/Identity/Copy) coexist in the
# "natural_log_exp_and_others" table set, but the table-load pass maps each
# func to the FIRST set containing it (exp -> set 0, ln -> set 5), forcing a
# ~2.7us table switch twice per tile.  Shrink the other sets' advertised
# membership so every func resolves to the one shared set -> a single load.
_ACT_FUNCS = {AF.Exp, AF.Ln, AF.Relu, AF.Identity, AF.Copy, AF.Square}
_ORIG_GAT = bacc.get_activation_tables


def _patched_gat(arch):
    tables = _ORIG_GAT(arch)
    return {
        name: (funcs if name == "natural_log_exp_and_others"
               else funcs - _ACT_FUNCS)
        for name, funcs in tables.items()
    }


bacc.get_activation_tables = _patched_gat



C = 256          # model dim
HID = 128        # demand-encoder hidden
NH = 8           # heads
HD = 32          # head dim
NCOL = 4         # collaborators
H, W = 128, 256
NCORES = 8
HSL = H // NCORES          # 16 rows of H per core
PPC = HSL * W              # 4096 pixels per core
TP = 256                   # pixels per tile (one H row)
NT = PPC // TP             # 16 tiles
NS = 1 + NCOL              # feature sources per pixel: ego + 4 collabs
# weight-blob column layout: 8 proj chunks + wqd2T + masks + wd1T
WBC = 9 * C + 72 + 1024 + HID


DEBUG_TAPS = False


def _build_program(has_bias: bool) -> bass.Bass:
    nc = bacc.Bacc("TRN2", target_bir_lowering=False, debug=False)

    feat_d = nc.dram_tensor("feat", [NT, 128, NS, 2, TP], BF, kind="ExternalInput")
    dem_d = nc.dram_tensor("demand", [3, PPC], BF, kind="ExternalInput")
    # All weights and masks ride in one DMA: [128, WBC] bf16, column-packed.
    wblob_d = nc.dram_tensor("wblob", [128, WBC], BF, kind="ExternalInput")
    if has_bias:
        bias_d = nc.dram_tensor("bias", [128, 5], F32, kind="ExternalInput")
    out_d = nc.dram_tensor("out", [NT, 128, 2, TP], BF, kind="ExternalOutput")
    if DEBUG_TAPS:
        dbg_q = nc.dram_tensor("dbg_q", [128, 2, TP], BF, kind="ExternalOutput")
        dbg_t0 = nc.dram_tensor("dbg_t0", [128, 2, TP], BF, kind="ExternalOutput")
        dbg_e = nc.dram_tensor("dbg_e", [128, TP], BF, kind="ExternalOutput")
        dbg_sc = nc.dram_tensor("dbg_sc", [128, TP], BF, kind="ExternalOutput")
        dbg_a0 = nc.dram_tensor("dbg_a0", [128, 2, TP], BF, kind="ExternalOutput")
        dbg_u = nc.dram_tensor("dbg_u", [128, 2, TP], BF, kind="ExternalOutput")
        dbg_k0 = nc.dram_tensor("dbg_k0", [128, 2, TP], F32, kind="ExternalOutput")

    with ExitStack() as ctx:
        tc = ctx.enter_context(tile.TileContext(nc))

        wp = ctx.enter_context(tc.tile_pool(name="wts", bufs=1))
        io = ctx.enter_context(tc.tile_pool(name="io", bufs=3))
        sp = ctx.enter_context(tc.tile_pool(name="sb", bufs=2))
        # PSUM: 8 banks.  qs{q,s2}=2 + o=1 + z{z,h}=2 + kv=3.
        pqs = ctx.enter_context(tc.tile_pool(name="pqs", bufs=2, space="PSUM"))
        po = ctx.enter_context(tc.tile_pool(name="po", bufs=1, space="PSUM"))
        pz = ctx.enter_context(tc.tile_pool(name="pz", bufs=2, space="PSUM"))
        pkv = ctx.enter_context(tc.tile_pool(name="pkv", bufs=3, space="PSUM"))

        # ---- prefetch the first feature tiles, then all weights in one DMA
        F_pre = {}

        def load(t):
            Ft = io.tile([128, NS, 2, TP], BF, tag="feat", bufs=5, name="F")
            nc.sync.dma_start(out=Ft, in_=feat_d[t])
            return Ft

        # Issue the four startup DMAs from four different engine queues so
        # their transfers overlap instead of serializing on one queue.
        F_pre[0] = load(0)
        F_pre[1] = load(1)
        dem_sb = wp.tile([3, PPC], BF, tag="dem", name="dem_sb")
        nc.gpsimd.dma_start(out=dem_sb, in_=dem_d[:])
        wb = wp.tile([128, WBC], BF, tag="wblob", name="wb")
        nc.scalar.dma_start(out=wb, in_=wblob_d[:])

        def wcol(off, n):
            return wb[:, off:off + n]

        # weight chunk [kc] restricted to output-channel chunk c
        def wqT(kc, c):
            return wcol(kc * C + 128 * c, 128)

        def wkT(kc, c):
            return wcol((2 + kc) * C + 128 * c, 128)

        def wvT(kc, c):
            return wcol((4 + kc) * C + 128 * c, 128)

        def woT(kc, c):
            return wcol((6 + kc) * C + 128 * c, 128)

        def wqd2T(c):
            return wcol(8 * C + 128 * c, 128)

        smask = [wcol(9 * C, 32), wcol(9 * C + 32, 32)]
        dmask = wcol(9 * C + 64, 8)
        zmask = [[wcol(9 * C + 72 + (2 * n + c) * 128, 128) for c in range(2)]
                 for n in range(NCOL)]
        wd1T = wb[0:3, 9 * C + 72 + 1024:9 * C + 72 + 1024 + HID]
        if has_bias:
            bias_sb = wp.tile([128, 5], F32, tag="bias", name="bias_sb")
            nc.sync.dma_start(out=bias_sb, in_=bias_d[:])

        # ---- demand-encoder hidden for the whole slab, once ----
        h_sb = wp.tile([128, PPC], BF, tag="h_sb", name="h_sb")
        HCH = 512
        for j in range(PPC // HCH):
            hx = ts(j, HCH)
            h_ps = pqs.tile([HID, HCH], F32, tag="qs", name="h_ps")
            nc.tensor.matmul(out=h_ps, lhsT=wd1T, rhs=dem_sb[:, hx],
                             start=True, stop=True)
            nc.scalar.activation(out=h_sb[:, hx], in_=h_ps, func=AF.Relu,
                                 bias=bias_sb[:, 0:1] if has_bias else 0.0)

        def front(t):
            """DMA + q/k projections + scores + softmax denominators."""
            F = F_pre.pop(t) if t in F_pre else load(t)

            # q projection (enc folded in via wqd2T)
            q_ps = pqs.tile([128, 2, TP], F32, tag="qs", name="q_ps")
            for c in range(2):
                nc.tensor.matmul(out=q_ps[:, c, :], lhsT=wqT(0, c),
                                 rhs=F[:, 0, 0, :], start=True, stop=False)
                nc.tensor.matmul(out=q_ps[:, c, :], lhsT=wqT(1, c),
                                 rhs=F[:, 0, 1, :], start=False, stop=False)
                nc.tensor.matmul(out=q_ps[:, c, :], lhsT=wqd2T(c),
                                 rhs=h_sb[:, ts(t, TP)], start=False, stop=True)
            q_sb = sp.tile([128, 2, TP], BF, tag="q", bufs=3, name="q_sb")
            if has_bias:
                for c in range(2):
                    nc.scalar.activation(out=q_sb[:, c, :], in_=q_ps[:, c, :],
                                         func=AF.Identity,
                                         bias=bias_sb[:, 1 + c:2 + c])
            else:
                nc.scalar.activation(out=q_sb, in_=q_ps, func=AF.Copy)

            # k projections pair-stacked: one 512-col matmul streams both
            # collabs of a pair for one output chunk, fully hiding the
            # LDWEIGHTS under the stream.  q is broadcast over the pair dim
            # in the DVE product.
            s2_ps = pqs.tile([128, TP], F32, tag="qs", name="s2_ps")
            t_pair = [None, None]   # per pair: [c] -> [128, 2(n), TP]

            def kpair(p):
                n0 = 2 * p
                kt = [pkv.tile([128, 2, TP], F32, tag="kv", name="k_ps")
                      for _ in range(2)]
                for c in range(2):
                    for kc in range(2):
                        nc.tensor.matmul(
                            out=kt[c], lhsT=wkT(kc, c),
                            rhs=F[:, 1 + n0:3 + n0, kc, :],
                            start=(kc == 0), stop=(kc == 1))
                tp = []
                for c in range(2):
                    tpc = sp.tile([128, 2, TP], BF, tag="t", bufs=6,
                                  name="t_sb")
                    qb = q_sb[:, c, :].unsqueeze(1).broadcast_to([128, 2, TP])
                    nc.vector.tensor_mul(tpc, qb, kt[c])
                    tp.append(tpc)
                t_pair[p] = tp

            def scores(p):
                for j in range(2):
                    n = 2 * p + j
                    for c in range(2):
                        nc.tensor.matmul(
                            out=s2_ps[32 * n:32 * n + 32, :], lhsT=smask[c],
                            rhs=t_pair[p][c][:, j, :],
                            start=(c == 0), stop=(c == 1),
                            tile_position=(0, 32 * n))

            kpair(0)
            kpair(1)
            scores(0)
            scores(1)

            # softmax prep: e = exp(s); den -> rows 0..7; L = ln(den)
            e2_sb = sp.tile([128, TP], BF, tag="e", bufs=3, name="e2_sb")
            nc.scalar.activation(out=e2_sb, in_=s2_ps, func=AF.Exp)
            s2c_sb = sp.tile([128, TP], BF, tag="sc", bufs=4, name="s2c_sb")
            nc.scalar.activation(out=s2c_sb, in_=s2_ps, func=AF.Copy)
            nc.tensor.matmul(out=s2_ps[0:8, :], lhsT=dmask, rhs=e2_sb,
                             start=True, stop=True)
            nc.scalar.activation(out=s2c_sb[0:8, :], in_=s2_ps[0:8, :],
                                 func=AF.Ln)
            if DEBUG_TAPS and t == 0:
                nc.sync.dma_start(out=dbg_q[:], in_=q_sb)
                nc.sync.dma_start(out=dbg_t0[:], in_=t_pair[0][0])
                nc.sync.dma_start(out=dbg_e[:], in_=e2_sb)
                nc.sync.dma_start(out=dbg_sc[:], in_=s2c_sb)
            return F, s2c_sb

        def mid(state, t):
            """Attention weights: z = s - L broadcast, a = exp(z).
            Two collabs share one 2-bank z tile so one exp covers both."""
            F, s2c_sb = state
            a_pair = [None, None]   # per pair: [128, 2(n), 2(c), TP]
            for p in range(2):
                z_ps = pz.tile([128, 2, 2, TP], F32, tag="z", bufs=1,
                               name="z_ps")
                for j in range(2):
                    for c in range(2):
                        nc.tensor.matmul(out=z_ps[:, j, c, :],
                                         lhsT=zmask[2 * p + j][c],
                                         rhs=s2c_sb, start=True, stop=True)
                ap = sp.tile([128, 2, 2, TP], BF, tag="a", bufs=6, name="a_sb")
                nc.scalar.activation(out=ap, in_=z_ps, func=AF.Exp)
                a_pair[p] = ap
            return F, a_pair

        def back(state, t):
            """v projections (pair-stacked), combine, output proj."""
            F, a_pair = state
            w_pc = [[None, None], [None, None]]   # [pair][c]

            def vpair(p):
                n0 = 2 * p
                vt = [pkv.tile([128, 2, TP], F32, tag="kv", name="v_ps")
                      for _ in range(2)]
                for c in range(2):
                    for kc in range(2):
                        nc.tensor.matmul(
                            out=vt[c], lhsT=wvT(kc, c),
                            rhs=F[:, 1 + n0:3 + n0, kc, :],
                            start=(kc == 0), stop=(kc == 1))
                for c in range(2):
                    wn = sp.tile([128, 2, TP], BF, tag="w", bufs=6,
                                 name="w_sb")
                    nc.vector.tensor_mul(wn, a_pair[p][:, :, c, :], vt[c])
                    w_pc[p][c] = wn

            vpair(0)
            vpair(1)
            # u[:, c, :] = sum over pairs and collabs of w; pairwise add in
            # n-space first (FD512), then fold the two collabs (FD256).
            u = sp.tile([128, 2, TP], BF, tag="u", bufs=3, name="u")
            for c in range(2):
                uu = sp.tile([128, 2, TP], BF, tag="uu", bufs=3, name="uu")
                nc.vector.tensor_add(uu, w_pc[0][c], w_pc[1][c])
                nc.vector.tensor_add(u[:, c, :], uu[:, 0, :], uu[:, 1, :])
            if DEBUG_TAPS and t == 0:
                nc.sync.dma_start(out=dbg_a0[:], in_=a_pair[0][:, 0])
                nc.sync.dma_start(out=dbg_u[:], in_=u)

            o_ps = po.tile([128, 2, TP], F32, tag="o", name="o_ps")
            for c in range(2):
                nc.tensor.matmul(out=o_ps[:, c, :], lhsT=woT(0, c),
                                 rhs=u[:, 0, :], start=True, stop=False)
                nc.tensor.matmul(out=o_ps[:, c, :], lhsT=woT(1, c),
                                 rhs=u[:, 1, :], start=False, stop=True)
            o_sb = io.tile([128, 2, TP], BF, tag="o_sb", bufs=3, name="o_sb")
            if has_bias:
                for c in range(2):
                    nc.scalar.activation(out=o_sb[:, c, :], in_=o_ps[:, c, :],
                                         func=AF.Identity,
                                         bias=bias_sb[:, 3 + c:4 + c])
            else:
                nc.scalar.activation(out=o_sb, in_=o_ps, func=AF.Copy)
            nc.sync.dma_start(out=out_d[t], in_=o_sb)

        # Three-stage software pipeline: emit front(t+2) and mid(t+1) before
        # back(t) so each engine's static in-order stream has two tiles of
        # independent work ahead of the current tile's dependency-stalled
        # tail.  The per-tile chain (PE->DVE->PE->Scalar->PE->Scalar->...)
        # is ~2 tile-periods long; 3 stages keep every engine fed.
        sF = [None] * NT   # front results
        sM = [None] * NT   # mid results
        sF[0] = front(0)
        sF[1] = front(1)
        sM[0] = mid(sF[0], 0)
        for t in range(NT):
            if t + 2 < NT:
                sF[t + 2] = front(t + 2)
            if t + 1 < NT:
                sM[t + 1] = mid(sF[t + 1], t + 1)
            back(sM[t], t)

    if not nc.is_finalized():
        nc.finalize()
    return nc


_PROGRAMS: dict[bool, bass.Bass] = {}


def _get_program(has_bias: bool) -> bass.Bass:
    if has_bias not in _PROGRAMS:
        _PROGRAMS[has_bias] = _build_program(has_bias)
    return _PROGRAMS[has_bias]


def _bf16(x):
    return np.asarray(x, dtype=np.float32).astype(ml_dtypes.bfloat16)


def _make_masks():
    # Score for (collab n, head g) sits at row 32n+8+g of the [128, 256]
    # score tile; rows 0..7 hold the softmax denominators (then ln of them).
    # Chunk c of the q*k product holds heads 4c+h' (h' = row//32).
    smask = np.zeros((2, 128, 32), np.float32)
    for c in range(2):
        for hp in range(4):
            smask[c, 32 * hp:32 * hp + 32, 8 + 4 * c + hp] = 1.0
    dmask = np.zeros((128, 8), np.float32)
    for n in range(NCOL):
        for g in range(8):
            dmask[32 * n + 8 + g, g] = 1.0
    # z_{n,c}[32h'+d] = s_n[head 4c+h'] - L[head 4c+h']
    zmask = np.zeros((NCOL, 2, 128, 128), np.float32)
    for n in range(NCOL):
        for c in range(2):
            for hp in range(4):
                g = 4 * c + hp
                zmask[n, c, 32 * n + 8 + g, 32 * hp:32 * hp + 32] = 1.0
                zmask[n, c, g, 32 * hp:32 * hp + 32] -= 1.0
    return _bf16(smask), _bf16(dmask), _bf16(zmask.reshape(NCOL * 2, 128, 128))


def _prepare(ego_features, ego_demand, collaborator_features,
             w_d1, b_d1, w_d2, b_d2, wq, bq, wk, bk, wv, bv, wo, bo,
             pos_emb):
    """Build (program, per-core input maps) for the SPMD launch."""
    ego_features = np.asarray(ego_features, np.float32)
    ego_demand = np.asarray(ego_demand, np.float32)
    collaborator_features = np.asarray(collaborator_features, np.float32)
    w_d1 = np.asarray(w_d1, np.float32); b_d1 = np.asarray(b_d1, np.float32)
    w_d2 = np.asarray(w_d2, np.float32); b_d2 = np.asarray(b_d2, np.float32)
    wq = np.asarray(wq, np.float32); bq = np.asarray(bq, np.float32)
    wk = np.asarray(wk, np.float32)
    wv = np.asarray(wv, np.float32); bv = np.asarray(bv, np.float32)
    wo = np.asarray(wo, np.float32); bo = np.asarray(bo, np.float32)
    pos_emb = np.asarray(pos_emb, np.float32)

    scale = 1.0 / math.sqrt(HD)
    wq_s = wq * scale
    wqd2 = wq_s @ w_d2                       # [C, HID]
    bq_eff = (bq + wq @ b_d2) * scale        # [C]
    bo_eff = bo + wo @ bv                    # [C]

    has_bias = bool(np.any(b_d1) or np.any(bq_eff) or np.any(bo_eff))
    nc = _get_program(has_bias)

    smask, dmask, zmask = _make_masks()
    wblob = np.zeros((128, WBC), ml_dtypes.bfloat16)
    wqTf = _bf16(wq_s.T.reshape(2, 128, C))
    wkTf = _bf16(wk.T.reshape(2, 128, C))
    wvTf = _bf16(wv.T.reshape(2, 128, C))
    woTf = _bf16(wo.T.reshape(2, 128, C))
    for kc in range(2):
        wblob[:, kc * C:(kc + 1) * C] = wqTf[kc]
        wblob[:, (2 + kc) * C:(3 + kc) * C] = wkTf[kc]
        wblob[:, (4 + kc) * C:(5 + kc) * C] = wvTf[kc]
        wblob[:, (6 + kc) * C:(7 + kc) * C] = woTf[kc]
    wblob[:, 8 * C:9 * C] = _bf16(wqd2.T)
    wblob[:, 9 * C:9 * C + 32] = smask[0]
    wblob[:, 9 * C + 32:9 * C + 64] = smask[1]
    wblob[:, 9 * C + 64:9 * C + 72] = dmask
    for i in range(8):
        wblob[:, 9 * C + 72 + i * 128:9 * C + 72 + (i + 1) * 128] = zmask[i]
    wblob[0:3, 9 * C + 72 + 1024:9 * C + 72 + 1024 + HID] = _bf16(w_d1.T)
    shared = {"wblob": wblob}
    if has_bias:
        bias = np.zeros((128, 5), np.float32)
        bias[:, 0] = b_d1
        bias[:, 1:3] = bq_eff.reshape(2, 128).T
        bias[:, 3:5] = bo_eff.reshape(2, 128).T
        shared["bias"] = bias

    # Global feature tensor [H, 128ch, NS, 2, W]; core i owns H rows
    # 16i..16i+16 (its NT=16 one-row tiles).  pos folded into ego here.
    ego_eff = ego_features[0]
    if np.any(pos_emb):
        ego_eff = ego_eff + pos_emb[0]
    feat = np.empty((H, 128, NS, 2, W), ml_dtypes.bfloat16)
    feat[:, :, 0] = _bf16(ego_eff).reshape(2, 128, H, W).transpose(2, 1, 0, 3)
    for n in range(NCOL):
        feat[:, :, 1 + n] = _bf16(collaborator_features[n]).reshape(
            2, 128, H, W).transpose(2, 1, 0, 3)

    dem_bf = _bf16(ego_demand[0])            # [3, H, W]

    in_maps = []
    for i in range(NCORES):
        m = dict(shared)
        m["feat"] = feat[HSL * i:HSL * (i + 1)]
        m["demand"] = np.ascontiguousarray(
            dem_bf[:, HSL * i:HSL * (i + 1), :].reshape(3, PPC))
        in_maps.append(m)
    return nc, in_maps


def _finish(res):
    """Gather per-core outputs back into the full [1, C, H, W] array."""
    oc = np.concatenate([res.results[i]["out"] for i in range(NCORES)], axis=0)
    # [H, 128ch, 2, W] -> [2, 128, H, W] -> [1, C, H, W]
    out = oc.transpose(2, 1, 0, 3).astype(np.float32).reshape(1, C, H, W)
    return np.ascontiguousarray(out)


def kernel(**inputs):
    nc, in_maps = _prepare(**inputs)
    res = run_bass_kernel_spmd(nc, in_maps, list(range(NCORES)))
    return _finish(res)


# revision 63
# speedup vs baseline: 1.0230x; 1.0230x over previous
"""Trainium2 Bass kernel for DemandAwareCrossAttention.

Reference computation (per pixel, fully pointwise in (H, W)):
    enc  = w_d2 @ relu(w_d1 @ demand + b_d1) + b_d2
    qs   = ego + enc + pos
    q    = (wq @ qs + bq)   reshaped [8 heads, 32]
    k_n  = wk @ collab_n + bk ; v_n = wv @ collab_n + bv     (n = 0..3)
    s_ng = q_g . k_ng / sqrt(32)
    a    = softmax_n(s)
    u    = sum_n a_ng * v_n            -> [256]
    out  = wo @ u + bo

Sharding: split H across the 8 cores (16 rows = 4096 pixels each); weights
replicated.  pos_emb is folded into ego on the host; bk cancels in the
softmax and is dropped; wq@b_d2+bq rides the q copy; wo@bv+bo rides the
output copy; q is pre-scaled by 1/sqrt(32) on host.

Device layout: channels on SBUF partitions, pixels on the free dim, channel
chunks c in {0,1} of 128.  One 256-pixel tile per H row.  Per tile:
  - ego + 4 collabs arrive as ONE contiguous DMA ([128, 5, 2, 256] bf16)
  - all 1x1 convs are PE matmuls (bf16, fp32 PSUM accumulate); the k/v
    projections are emitted in lhsT-grouped runs so the PE streams
    back-to-back (keeps the HAM clock gate at 2.4 GHz)
  - scores: DVE q*k product, then masked matmuls pack score(n, head g) at
    PSUM row 32n+8+g of a [128, 256] tile (both chunks' heads in one
    256-wide tile - halves the free dim every softmax op pays for)
  - divide-free softmax: e = exp(s) (ScalarE), denom via masked matmul
    into rows 0..7, L = ln(denom) into the copy's rows 0..7, then masked
    matmuls form z = s - L broadcast over head_dim, a = exp(z)
  - combine u = sum_n a_n * v_n on GpSimd (muls + adds); q/out PSUM->SBUF
    copies (+ bias) on DVE; exp/ln stay on ScalarE
  - output stored as bf16 (halves the writeback)

Engine budget per tile (target):  PE ~59 matmuls of 256 cols; ScalarE
exp/copy/ln + 4 big exps; DVE q-copy + 4 muls + o-copy; GpSimd 4 muls +
3 adds; one DMA in, one out.
"""

import math
import numpy as np
import ml_dtypes
from contextlib import ExitStack

import concourse.bass as bass
import concourse.tile as tile
from concourse import bacc, mybir
from concourse.bass import ts
from concourse.bass_utils import run_bass_kernel_spmd

BF = mybir.dt.bfloat16
F32 = mybir.dt.float32
AF = mybir.ActivationFunctionType

# All ScalarE functions used here (Exp/Ln/Relu/Identity/Copy) coexist in the
# "natural_log_exp_and_others" table set, but the table-load pass maps each
# func to the FIRST set containing it (exp -> set 0, ln -> set 5), forcing a
# ~2.7us table switch twice per tile.  Shrink the other sets' advertised
# membership so every func resolves to the one shared set -> a single load.
_ACT_FUNCS = {AF.Exp, AF.Ln, AF.Relu, AF.Identity, AF.Copy, AF.Square}
_ORIG_GAT = bacc.get_activation_tables


def _patched_gat(arch):
    tables = _ORIG_GAT(arch)
    return {
        name: (funcs if name == "natural_log_exp_and_others"
               else funcs - _ACT_FUNCS)
        for name, funcs in tables.items()
    }


bacc.get_activation_tables = _patched_gat



C = 256          # model dim
HID = 128        # demand-encoder hidden
NH = 8           # heads
HD = 32          # head dim
NCOL = 4         # collaborators
H, W = 128, 256
NCORES = 8
HSL = H // NCORES          # 16 rows of H per core
PPC = HSL * W              # 4096 pixels per core
TP = 256                   # pixels per tile (one H row)
NT = PPC // TP             # 16 tiles
NS = 1 + NCOL              # feature sources per pixel: ego + 4 collabs
# weight-blob column layout: 8 proj chunks + wqd2T + masks + wd1T
WBC = 9 * C + 72 + 1024 + HID


DEBUG_TAPS = False


def _build_program(has_bias: bool) -> bass.Bass:
    nc = bacc.Bacc("TRN2", target_bir_lowering=False, debug=False)

    feat_d = nc.dram_tensor("feat", [NT, 128, NS, 2, TP], BF, kind="ExternalInput")
    dem_d = nc.dram_tensor("demand", [3, PPC], BF, kind="ExternalInput")
    # All weights and masks ride in one DMA: [128, WBC] bf16, column-packed.
    wblob_d = nc.dram_tensor("wblob", [128, WBC], BF, kind="ExternalInput")
    if has_bias:
        bias_d = nc.dram_tensor("bias", [128, 5], F32, kind="ExternalInput")
    out_d = nc.dram_tensor("out", [NT, 128, 2, TP], BF, kind="ExternalOutput")
    if DEBUG_TAPS:
        dbg_q = nc.dram_tensor("dbg_q", [128, 2, TP], BF, kind="ExternalOutput")
        dbg_t0 = nc.dram_tensor("dbg_t0", [128, 2, TP], BF, kind="ExternalOutput")
        dbg_e = nc.dram_tensor("dbg_e", [128, TP], BF, kind="ExternalOutput")
        dbg_sc = nc.dram_tensor("dbg_sc", [128, TP], BF, kind="ExternalOutput")
        dbg_a0 = nc.dram_tensor("dbg_a0", [128, 2, TP], BF, kind="ExternalOutput")
        dbg_u = nc.dram_tensor("dbg_u", [128, 2, TP], BF, kind="ExternalOutput")
        dbg_k0 = nc.dram_tensor("dbg_k0", [128, 2, TP], F32, kind="ExternalOutput")

    with ExitStack() as ctx:
        tc = ctx.enter_context(tile.TileContext(nc))

        wp = ctx.enter_context(tc.tile_pool(name="wts", bufs=1))
        io = ctx.enter_context(tc.tile_pool(name="io", bufs=3))
        sp = ctx.enter_context(tc.tile_pool(name="sb", bufs=2))
        # PSUM: 8 banks.  qs{q,s2}=2 + o=1 + z{z,h}=2 + kv=3.
        pqs = ctx.enter_context(tc.tile_pool(name="pqs", bufs=2, space="PSUM"))
        po = ctx.enter_context(tc.tile_pool(name="po", bufs=1, space="PSUM"))
        pz = ctx.enter_context(tc.tile_pool(name="pz", bufs=2, space="PSUM"))
        pkv = ctx.enter_context(tc.tile_pool(name="pkv", bufs=3, space="PSUM"))

        # ---- prefetch the first feature tiles, then all weights in one DMA
        F_pre = {}

        def load(t):
            Ft = io.tile([128, NS, 2, TP], BF, tag="feat", bufs=4, name="F")
            nc.sync.dma_start(out=Ft, in_=feat_d[t])
            return Ft

        # Issue the four startup DMAs from four different engine queues so
        # their transfers overlap instead of serializing on one queue.
        F_pre[0] = load(0)
        F_pre[1] = load(1)
        dem_sb = wp.tile([3, PPC], BF, tag="dem", name="dem_sb")
        nc.gpsimd.dma_start(out=dem_sb, in_=dem_d[:])
        wb = wp.tile([128, WBC], BF, tag="wblob", name="wb")
        nc.scalar.dma_start(out=wb, in_=wblob_d[:])

        def wcol(off, n):
            return wb[:, off:off + n]

        # weight chunk [kc] restricted to output-channel chunk c
        def wqT(kc, c):
            return wcol(kc * C + 128 * c, 128)

        def wkT(kc, c):
            return wcol((2 + kc) * C + 128 * c, 128)

        def wvT(kc, c):
            return wcol((4 + kc) * C + 128 * c, 128)

        def woT(kc, c):
            return wcol((6 + kc) * C + 128 * c, 128)

        def wqd2T(c):
            return wcol(8 * C + 128 * c, 128)

        smask = [wcol(9 * C, 32), wcol(9 * C + 32, 32)]
        dmask = wcol(9 * C + 64, 8)
        zmask = [[wcol(9 * C + 72 + (2 * n + c) * 128, 128) for c in range(2)]
                 for n in range(NCOL)]
        wd1T = wb[0:3, 9 * C + 72 + 1024:9 * C + 72 + 1024 + HID]
        if has_bias:
            bias_sb = wp.tile([128, 5], F32, tag="bias", name="bias_sb")
            nc.sync.dma_start(out=bias_sb, in_=bias_d[:])

        # ---- demand-encoder hidden for the whole slab, once ----
        h_sb = wp.tile([128, PPC], BF, tag="h_sb", name="h_sb")
        HCH = 512
        for j in range(PPC // HCH):
            hx = ts(j, HCH)
            h_ps = pqs.tile([HID, HCH], F32, tag="qs", name="h_ps")
            nc.tensor.matmul(out=h_ps, lhsT=wd1T, rhs=dem_sb[:, hx],
                             start=True, stop=True)
            nc.scalar.activation(out=h_sb[:, hx], in_=h_ps, func=AF.Relu,
                                 bias=bias_sb[:, 0:1] if has_bias else 0.0)

        def front(t):
            """DMA + q/k projections + scores + softmax denominators."""
            F = F_pre.pop(t) if t in F_pre else load(t)

            # q projection (enc folded in via wqd2T)
            q_ps = pqs.tile([128, 2, TP], F32, tag="qs", name="q_ps")
            for c in range(2):
                nc.tensor.matmul(out=q_ps[:, c, :], lhsT=wqT(0, c),
                                 rhs=F[:, 0, 0, :], start=True, stop=False)
                nc.tensor.matmul(out=q_ps[:, c, :], lhsT=wqT(1, c),
                                 rhs=F[:, 0, 1, :], start=False, stop=False)
                nc.tensor.matmul(out=q_ps[:, c, :], lhsT=wqd2T(c),
                                 rhs=h_sb[:, ts(t, TP)], start=False, stop=True)
            q_sb = sp.tile([128, 2, TP], BF, tag="q", name="q_sb")
            if has_bias:
                for c in range(2):
                    nc.scalar.activation(out=q_sb[:, c, :], in_=q_ps[:, c, :],
                                         func=AF.Identity,
                                         bias=bias_sb[:, 1 + c:2 + c])
            else:
                nc.scalar.activation(out=q_sb, in_=q_ps, func=AF.Copy)

            # k projections pair-stacked: one 512-col matmul streams both
            # collabs of a pair for one output chunk, fully hiding the
            # LDWEIGHTS under the stream.  q is broadcast over the pair dim
            # in the DVE product.
            s2_ps = pqs.tile([128, TP], F32, tag="qs", name="s2_ps")
            t_pair = [None, None]   # per pair: [c] -> [128, 2(n), TP]

            def kpair(p):
                n0 = 2 * p
                kt = [pkv.tile([128, 2, TP], F32, tag="kv", name="k_ps")
                      for _ in range(2)]
                for c in range(2):
                    for kc in range(2):
                        nc.tensor.matmul(
                            out=kt[c], lhsT=wkT(kc, c),
                            rhs=F[:, 1 + n0:3 + n0, kc, :],
                            start=(kc == 0), stop=(kc == 1))
                tp = []
                for c in range(2):
                    tpc = sp.tile([128, 2, TP], BF, tag="t", bufs=4,
                                  name="t_sb")
                    qb = q_sb[:, c, :].unsqueeze(1).broadcast_to([128, 2, TP])
                    nc.vector.tensor_mul(tpc, qb, kt[c])
                    tp.append(tpc)
                t_pair[p] = tp

            def scores(p):
                for j in range(2):
                    n = 2 * p + j
                    for c in range(2):
                        nc.tensor.matmul(
                            out=s2_ps[32 * n:32 * n + 32, :], lhsT=smask[c],
                            rhs=t_pair[p][c][:, j, :],
                            start=(c == 0), stop=(c == 1),
                            tile_position=(0, 32 * n))

            kpair(0)
            kpair(1)
            scores(0)
            scores(1)

            # softmax prep: e = exp(s); den -> rows 0..7; L = ln(den)
            e2_sb = sp.tile([128, TP], BF, tag="e", name="e2_sb")
            nc.scalar.activation(out=e2_sb, in_=s2_ps, func=AF.Exp)
            s2c_sb = sp.tile([128, TP], BF, tag="sc", bufs=3, name="s2c_sb")
            nc.scalar.activation(out=s2c_sb, in_=s2_ps, func=AF.Copy)
            nc.tensor.matmul(out=s2_ps[0:8, :], lhsT=dmask, rhs=e2_sb,
                             start=True, stop=True)
            nc.scalar.activation(out=s2c_sb[0:8, :], in_=s2_ps[0:8, :],
                                 func=AF.Ln)
            if DEBUG_TAPS and t == 0:
                nc.sync.dma_start(out=dbg_q[:], in_=q_sb)
                nc.sync.dma_start(out=dbg_t0[:], in_=t_pair[0][0])
                nc.sync.dma_start(out=dbg_e[:], in_=e2_sb)
                nc.sync.dma_start(out=dbg_sc[:], in_=s2c_sb)
            return F, s2c_sb

        def mid(state, t):
            """Attention weights: z = s - L broadcast, a = exp(z).
            Two collabs share one 2-bank z tile so one exp covers both."""
            F, s2c_sb = state
            a_pair = [None, None]   # per pair: [128, 2(n), 2(c), TP]
            for p in range(2):
                z_ps = pz.tile([128, 2, 2, TP], F32, tag="z", bufs=1,
                               name="z_ps")
                for j in range(2):
                    for c in range(2):
                        nc.tensor.matmul(out=z_ps[:, j, c, :],
                                         lhsT=zmask[2 * p + j][c],
                                         rhs=s2c_sb, start=True, stop=True)
                ap = sp.tile([128, 2, 2, TP], BF, tag="a", bufs=4, name="a_sb")
                nc.scalar.activation(out=ap, in_=z_ps, func=AF.Exp)
                a_pair[p] = ap
            return F, a_pair

        def back(state, t):
            """v projections (pair-stacked), combine, output proj."""
            F, a_pair = state
            w_pc = [[None, None], [None, None]]   # [pair][c]

            def vpair(p):
                n0 = 2 * p
                vt = [pkv.tile([128, 2, TP], F32, tag="kv", name="v_ps")
                      for _ in range(2)]
                for c in range(2):
                    for kc in range(2):
                        nc.tensor.matmul(
                            out=vt[c], lhsT=wvT(kc, c),
                            rhs=F[:, 1 + n0:3 + n0, kc, :],
                            start=(kc == 0), stop=(kc == 1))
                for c in range(2):
                    wn = sp.tile([128, 2, TP], BF, tag="w", bufs=4,
                                 name="w_sb")
                    nc.vector.tensor_mul(wn, a_pair[p][:, :, c, :], vt[c])
                    w_pc[p][c] = wn

            vpair(0)
            vpair(1)
            # u[:, c, :] = sum over pairs and collabs of w; pairwise add in
            # n-space first (FD512), then fold the two collabs (FD256).
            u = sp.tile([128, 2, TP], BF, tag="u", bufs=2, name="u")
            for c in range(2):
                uu = sp.tile([128, 2, TP], BF, tag="uu", bufs=2, name="uu")
                nc.vector.tensor_add(uu, w_pc[0][c], w_pc[1][c])
                nc.vector.tensor_add(u[:, c, :], uu[:, 0, :], uu[:, 1, :])
            if DEBUG_TAPS and t == 0:
                nc.sync.dma_start(out=dbg_a0[:], in_=a_pair[0][:, 0])
                nc.sync.dma_start(out=dbg_u[:], in_=u)

            o_ps = po.tile([128, 2, TP], F32, tag="o", name="o_ps")
            for c in range(2):
                nc.tensor.matmul(out=o_ps[:, c, :], lhsT=woT(0, c),
                                 rhs=u[:, 0, :], start=True, stop=False)
                nc.tensor.matmul(out=o_ps[:, c, :], lhsT=woT(1, c),
                                 rhs=u[:, 1, :], start=False, stop=True)
            o_sb = io.tile([128, 2, TP], BF, tag="o_sb", bufs=2, name="o_sb")
            if has_bias:
                for c in range(2):
                    nc.scalar.activation(out=o_sb[:, c, :], in_=o_ps[:, c, :],
                                         func=AF.Identity,
                                         bias=bias_sb[:, 3 + c:4 + c])
            else:
                nc.scalar.activation(out=o_sb, in_=o_ps, func=AF.Copy)
            nc.sync.dma_start(out=out_d[t], in_=o_sb)

        # Three-stage software pipeline: emit front(t+2) and mid(t+1) before
        # back(t) so each engine's static in-order stream has two tiles of
        # independent work ahead of the current tile's dependency-stalled
        # tail.  The per-tile chain (PE->DVE->PE->Scalar->PE->Scalar->...)
        # is ~2 tile-periods long; 3 stages keep every engine fed.
        sF = [None] * NT   # front results
        sM = [None] * NT   # mid results
        sF[0] = front(0)
        sF[1] = front(1)
        sM[0] = mid(sF[0], 0)
        for t in range(NT):
            if t + 2 < NT:
                sF[t + 2] = front(t + 2)
            if t + 1 < NT:
                sM[t + 1] = mid(sF[t + 1], t + 1)
            back(sM[t], t)

    if not nc.is_finalized():
        nc.finalize()
    return nc


_PROGRAMS: dict[bool, bass.Bass] = {}


def _get_program(has_bias: bool) -> bass.Bass:
    if has_bias not in _PROGRAMS:
        _PROGRAMS[has_bias] = _build_program(has_bias)
    return _PROGRAMS[has_bias]


def _bf16(x):
    return np.asarray(x, dtype=np.float32).astype(ml_dtypes.bfloat16)


def _make_masks():
    # Score for (collab n, head g) sits at row 32n+8+g of the [128, 256]
    # score tile; rows 0..7 hold the softmax denominators (then ln of them).
    # Chunk c of the q*k product holds heads 4c+h' (h' = row//32).
    smask = np.zeros((2, 128, 32), np.float32)
    for c in range(2):
        for hp in range(4):
            smask[c, 32 * hp:32 * hp + 32, 8 + 4 * c + hp] = 1.0
    dmask = np.zeros((128, 8), np.float32)
    for n in range(NCOL):
        for g in range(8):
            dmask[32 * n + 8 + g, g] = 1.0
    # z_{n,c}[32h'+d] = s_n[head 4c+h'] - L[head 4c+h']
    zmask = np.zeros((NCOL, 2, 128, 128), np.float32)
    for n in range(NCOL):
        for c in range(2):
            for hp in range(4):
                g = 4 * c + hp
                zmask[n, c, 32 * n + 8 + g, 32 * hp:32 * hp + 32] = 1.0
                zmask[n, c, g, 32 * hp:32 * hp + 32] -= 1.0
    return _bf16(smask), _bf16(dmask), _bf16(zmask.reshape(NCOL * 2, 128, 128))


def _prepare(ego_features, ego_demand, collaborator_features,
             w_d1, b_d1, w_d2, b_d2, wq, bq, wk, bk, wv, bv, wo, bo,
             pos_emb):
    """Build (program, per-core input maps) for the SPMD launch."""
    ego_features = np.asarray(ego_features, np.float32)
    ego_demand = np.asarray(ego_demand, np.float32)
    collaborator_features = np.asarray(collaborator_features, np.float32)
    w_d1 = np.asarray(w_d1, np.float32); b_d1 = np.asarray(b_d1, np.float32)
    w_d2 = np.asarray(w_d2, np.float32); b_d2 = np.asarray(b_d2, np.float32)
    wq = np.asarray(wq, np.float32); bq = np.asarray(bq, np.float32)
    wk = np.asarray(wk, np.float32)
    wv = np.asarray(wv, np.float32); bv = np.asarray(bv, np.float32)
    wo = np.asarray(wo, np.float32); bo = np.asarray(bo, np.float32)
    pos_emb = np.asarray(pos_emb, np.float32)

    scale = 1.0 / math.sqrt(HD)
    wq_s = wq * scale
    wqd2 = wq_s @ w_d2                       # [C, HID]
    bq_eff = (bq + wq @ b_d2) * scale        # [C]
    bo_eff = bo + wo @ bv                    # [C]

    has_bias = bool(np.any(b_d1) or np.any(bq_eff) or np.any(bo_eff))
    nc = _get_program(has_bias)

    smask, dmask, zmask = _make_masks()
    wblob = np.zeros((128, WBC), ml_dtypes.bfloat16)
    wqTf = _bf16(wq_s.T.reshape(2, 128, C))
    wkTf = _bf16(wk.T.reshape(2, 128, C))
    wvTf = _bf16(wv.T.reshape(2, 128, C))
    woTf = _bf16(wo.T.reshape(2, 128, C))
    for kc in range(2):
        wblob[:, kc * C:(kc + 1) * C] = wqTf[kc]
        wblob[:, (2 + kc) * C:(3 + kc) * C] = wkTf[kc]
        wblob[:, (4 + kc) * C:(5 + kc) * C] = wvTf[kc]
        wblob[:, (6 + kc) * C:(7 + kc) * C] = woTf[kc]
    wblob[:, 8 * C:9 * C] = _bf16(wqd2.T)
    wblob[:, 9 * C:9 * C + 32] = smask[0]
    wblob[:, 9 * C + 32:9 * C + 64] = smask[1]
    wblob[:, 9 * C + 64:9 * C + 72] = dmask
    for i in range(8):
        wblob[:, 9 * C + 72 + i * 128:9 * C + 72 + (i + 1) * 128] = zmask[i]
    wblob[0:3, 9 * C + 72 + 1024:9 * C + 72 + 1024 + HID] = _bf16(w_d1.T)
    shared = {"wblob": wblob}
    if has_bias:
        bias = np.zeros((128, 5), np.float32)
        bias[:, 0] = b_d1
        bias[:, 1:3] = bq_eff.reshape(2, 128).T
        bias[:, 3:5] = bo_eff.reshape(2, 128).T
        shared["bias"] = bias

    # Global feature tensor [H, 128ch, NS, 2, W]; core i owns H rows
    # 16i..16i+16 (its NT=16 one-row tiles).  pos folded into ego here.
    ego_eff = ego_features[0]
    if np.any(pos_emb):
        ego_eff = ego_eff + pos_emb[0]
    feat = np.empty((H, 128, NS, 2, W), ml_dtypes.bfloat16)
    feat[:, :, 0] = _bf16(ego_eff).reshape(2, 128, H, W).transpose(2, 1, 0, 3)
    for n in range(NCOL):
        feat[:, :, 1 + n] = _bf16(collaborator_features[n]).reshape(
            2, 128, H, W).transpose(2, 1, 0, 3)

    dem_bf = _bf16(ego_demand[0])            # [3, H, W]

    in_maps = []
    for i in range(NCORES):
        m = dict(shared)
        m["feat"] = feat[HSL * i:HSL * (i + 1)]
        m["demand"] = np.ascontiguousarray(
            dem_bf[:, HSL * i:HSL * (i + 1), :].reshape(3, PPC))
        in_maps.append(m)
    return nc, in_maps


def _finish(res):
    """Gather per-core outputs back into the full [1, C, H, W] array."""
    oc = np.concatenate([res.results[i]["out"] for i in range(NCORES)], axis=0)
    # [H, 128ch, 2, W] -> [2, 128, H, W] -> [1, C, H, W]
    out = oc.transpose(2, 1, 0, 3).astype(np.float32).reshape(1, C, H, W)
    return np.ascontiguousarray(out)


def kernel(**inputs):
    nc, in_maps = _prepare(**inputs)
    res = run_bass_kernel_spmd(nc, in_maps, list(range(NCORES)))
    return _finish(res)
